# revision 1
# baseline (speedup 1.0000x reference)
"""Trainium2 Bass kernel for additive-attention pooling.

Computation (per batch row b):
    Wah   = h @ Wah_w.T                         [B, HID]
    e     = tanh(Wah[:, None, :] + p_att_feats) [B, L, HID]
    s     = e @ alpha_w[0]                      [B, L]
    alpha = softmax(s, -1)                      [B, L]
    att   = sum_l alpha[b, l] * att_feats[b, l, :]   [B, FEAT]

Sharding: pure data parallel over the batch dim, 32 rows per core on 8
NeuronCores; the small Wah_w / alpha_w weights are replicated.

Per-core dataflow. att_feats / p_att_feats are staged to device HBM in
bf16 (the TRN2 native compute dtype; fp32 PSUM accumulation keeps the
result within ~3e-3 of the fp32 reference), host-packed per PAIR of
batch rows as [pair, 208, 2, width] so every input DMA is a 128- or
80-partition fully contiguous block (sub-128-partition or strided DMAs
fan out to fewer SDMA engines and run far below peak). h and Wah_w are
staged host-transposed (r-major) so the device does no weight
transposes.

  setup : WahT[h, b] = sum_r Wah_w[h, r] h[b, r] accumulated over
          r-chunks in 4 PSUM banks; alpha_w transposed via K=1 matmuls.
  phase1: stream p_att pair tiles -> PE transpose to [h, l] (bf16)
          -> ScalarE fused bias(=WahT column)+tanh -> bf16 e tiles
          -> TensorE contracts h with alpha_w -> scores [1, 2, L]
          -> ScalarE exp with fused row-sum (accum_out) -> reciprocal
          -> two K=1 matmuls transpose each alpha row into a [L, 1]
          column, folding the 1/sum normalization into the matmul rhs.
  phase2: stream att_feats pair tiles -> per batch 8 matvec matmuls
          (bf16, K = 128/68 l-chunks, N = 512 per PSUM bank) -> copies
          to a flat partition-0 staging row -> one output DMA per pair
          on the otherwise idle GpSimd (SWDGE) queue.

The walrus build in this image accepts only one semaphore wait and one
update per instruction; _split_sync() post-processes the scheduled BIR
to spread Tile's multi-wait/multi-update sync info onto NoOp carriers.
"""

import os
import sys
import types

sys.path.insert(0, "/opt/trn_rl_repo")

# This image's antenv package lacks axon_hooks; provide it so
# concourse.bass_utils can import it (trace path) without crashing.
if "antenv.axon_hooks" not in sys.modules:
    _m = types.ModuleType("antenv.axon_hooks")

    def _set_hook(h):
        _m._hook = h

    def _get_hook():
        return getattr(_m, "_hook", None)

    _m.set_axon_ntff_profile_hook = _set_hook
    _m.get_axon_ntff_profile_hook = _get_hook
    sys.modules["antenv.axon_hooks"] = _m
    import antenv

    antenv.axon_hooks = _m

import numpy as np  # noqa: E402
import bass_rust  # noqa: E402
import concourse.bass as bass  # noqa: E402
import concourse.tile as tile  # noqa: E402
from concourse import mybir  # noqa: E402
from concourse.masks import make_identity  # noqa: E402
from concourse.tile_rust import add_dep_helper  # noqa: E402

F32 = mybir.dt.float32
F32R = mybir.dt.float32r
BF16 = mybir.dt.bfloat16
PSUM = bass.MemorySpace.PSUM
Tanh = mybir.ActivationFunctionType.Tanh
Exp = mybir.ActivationFunctionType.Exp

B, L, RNN, HID, FEAT = 256, 196, 1024, 512, 2048
NCORES = 8
BL = B // NCORES  # batch rows per core
L_HI = 128
L_LO = L - L_HI  # 68
LPAD = 208  # l rows padded to a multiple of 16 for full SDMA fan-out
NHC = HID // 128  # h chunks
NRC = RNN // 128  # r chunks
NFQ = FEAT // 512  # psum-bank-sized f chunks
NPAIR = BL // 2

AF_BUFS = int(os.environ.get("KERNEL_AF_BUFS", "4"))


def _split_sync(nc):
    """walrus in this image encodes at most ONE semaphore wait and ONE
    semaphore update per instruction; Tile freely emits several. Move the
    extras onto single-wait/single-update NoOp carriers on the same engine
    (engine queues are strict FIFO, so a preceding NoOp's wait gates the
    instruction and a following NoOp's update fires after it completes)."""
    dma_types = {
        "InstDMACopy",
        "InstTensorLoad",
        "InstTensorSave",
        "InstDmaTransposeAnt",
        "InstTensorCopy",
    }
    for f in nc.m.functions:
        for bb in f.blocks:
            new = []
            changed = False
            for ins in bb.instructions:
                si = ins.sync_info
                if si is None:
                    new.append(ins)
                    continue
                waits = list(si.on_wait)
                updates = list(si.on_update)
                if len(waits) <= 1 and len(updates) <= 1:
                    new.append(ins)
                    continue
                changed = True
                tname = type(ins).__name__
                for j, w in enumerate(waits[:-1]):
                    nop = mybir.InstNoOp(name=f"{ins.name}_w{j}", ins=[], outs=[])
                    nop.engine = ins.engine
                    nop.sync_info = bass_rust.SyncInfo(on_wait=[w], on_update=[])
                    new.append(nop)
                keep_w = waits[-1:]
                post_u = []
                keep_u = updates
                if len(updates) > 1:
                    if tname in dma_types:
                        raise RuntimeError(
                            f"DMA instruction {ins.name} carries {len(updates)} "
                            "sem updates; cannot split without changing semantics"
                        )
                    keep_u = updates[:1]
                    post_u = updates[1:]
                ins.sync_info = bass_rust.SyncInfo(on_wait=keep_w, on_update=keep_u)
                new.append(ins)
                for j, u in enumerate(post_u):
                    nop = mybir.InstNoOp(name=f"{ins.name}_u{j}", ins=[], outs=[])
                    nop.engine = ins.engine
                    nop.sync_info = bass_rust.SyncInfo(on_wait=[], on_update=[u])
                    new.append(nop)
            if changed:
                bb.instructions = new


def build_nc(split=True):
    """Inputs arrive host-packed per pair of batch rows:
      att_feats: [NPAIR, LPAD, 2, FEAT] bf16 (l rows 196..207 zero)
      p_att_feats: [NPAIR, LPAD, 2, HID] bf16
    so every input DMA is a 128- or 80-partition fully-contiguous block."""
    nc = bass.Bass()
    h_d = nc.declare_dram_parameter("h", [RNN, BL], F32, isOutput=False)
    af_d = nc.declare_dram_parameter(
        "att_feats", [NPAIR, LPAD, 2, FEAT], BF16, isOutput=False
    )
    pa_d = nc.declare_dram_parameter(
        "p_att_feats", [NPAIR, LPAD, 2, HID], BF16, isOutput=False
    )
    ww_d = nc.declare_dram_parameter("Wah_w", [RNN, HID], F32, isOutput=False)
    aw_d = nc.declare_dram_parameter("alpha_w", [1, HID], F32, isOutput=False)
    out_d = nc.declare_dram_parameter("out", [BL, FEAT], F32, isOutput=True)

    with tile.TileContext(nc) as tc:
        with tc.tile_pool(name="singles", bufs=1) as singles:
            # bf16 identity for the bf16 p_att transposes (0/1 exact in bf16)
            identity_bf = singles.tile([128, 128], BF16)
            make_identity(nc, identity_bf[:])
            wahT = singles.tile([128, NHC, BL], F32)  # WahT[h % 128, hc, b]
            awT = singles.tile([128, NHC], BF16)  # alpha_w^T chunks
            # exp(scores): 256-wide zero-padded slot per batch so the two
            # alphaT transpose matmuls both span 128 output partitions (the
            # PSUM accumulation-group bookkeeping is per-partition)
            LP = 256
            expS = singles.tile([1, BL * LP], F32)
            nc.gpsimd.memset(expS[:], 0.0)
            sums = singles.tile([1, BL], F32)
            rsum = singles.tile([1, BL], F32)
            aT_sb = singles.tile([128, BL, 2], BF16)  # alphaT cols (hi, lo)

            # Batch-loop SBUF pools are allocated FIRST so their zones never
            # overlap the setup pool's — otherwise the first p_att/att_feats
            # DMAs inherit released-zone deps on the whole setup computation
            # and the input stream idles ~25us at the head.
            with (
                tc.tile_pool(name="pat", bufs=6) as pool_pat,
                tc.tile_pool(name="af", bufs=AF_BUFS) as pool_af,
                tc.tile_pool(name="e", bufs=6) as pool_e,
                tc.tile_pool(name="ob", bufs=2) as pool_ob,
            ):
                # ---------------- setup: weights ----------------
                # h and Wah_w arrive host-transposed (r-major), so WahT is a
                # plain accumulated matmul with no on-chip transposes.
                with (
                    tc.tile_pool(name="setup_sb", bufs=1) as ssb,
                    tc.tile_pool(name="setup_ps", bufs=2, space=PSUM) as sps,
                    tc.tile_pool(name="setup_acc", bufs=1, space=PSUM) as sacc,
                ):
                    hT = ssb.tile([128, NRC, BL], F32)
                    nc.sync.dma_start(
                        hT[:], h_d[:].rearrange("(rc p) b -> p rc b", p=128)
                    )
                    wwT = ssb.tile([128, NRC, HID], F32)
                    nc.sync.dma_start(
                        wwT[:], ww_d[:].rearrange("(rc p) c -> p rc c", p=128)
                    )
                    aw_sb = ssb.tile([1, HID], F32)
                    nc.sync.dma_start(aw_sb[:], aw_d[:])
                    ones11 = ssb.tile([1, 1], F32)
                    nc.gpsimd.memset(ones11[:], 1.0)

                    # alpha_w^T columns (bf16 to match bf16 e tiles)
                    for hc in range(NHC):
                        ps = sps.tile([128, 1], F32, tag="aw")
                        nc.tensor.matmul(
                            ps[:],
                            aw_sb[0:1, hc * 128 : (hc + 1) * 128],
                            ones11[:],
                            start=True,
                            stop=True,
                        )
                        nc.vector.tensor_copy(awT[:, hc : hc + 1], ps[:])

                    # WahT[h, b] = sum_r Wah_w[h, r] * h[b, r]
                    wahT_ps = [
                        sacc.tile([128, BL], F32, tag=f"acc{hc}", name=f"wahT_ps{hc}")
                        for hc in range(NHC)
                    ]
                    for rc in range(NRC):
                        for hc in range(NHC):
                            nc.tensor.matmul(
                                wahT_ps[hc][:],
                                wwT[:, rc, hc * 128 : (hc + 1) * 128],
                                hT[:, rc, :],
                                start=(rc == 0),
                                stop=(rc == NRC - 1),
                            )
                    for hc in range(NHC):
                        nc.vector.tensor_copy(wahT[:, hc, :], wahT_ps[hc][:])

                # ---------------- streaming batch loop ----------------
                with (
                    tc.tile_pool(name="tp_ps", bufs=3, space=PSUM) as pool_tp,
                    tc.tile_pool(name="sc_ps", bufs=1, space=PSUM) as pool_sc,
                    tc.tile_pool(name="aT_ps", bufs=1, space=PSUM) as pool_aT,
                    tc.tile_pool(name="ao_ps", bufs=3, space=PSUM) as pool_ao,
                ):
                    prev_aT_read = None
                    for p in range(NPAIR):
                        b0 = 2 * p
                        pa_hi = pool_pat.tile([L_HI, 2, HID], BF16, tag="pa_hi")
                        nc.sync.dma_start(pa_hi[:], pa_d[p, 0:L_HI])
                        pa_lo = pool_pat.tile(
                            [LPAD - L_HI, 2, HID], BF16, tag="pa_lo"
                        )
                        nc.sync.dma_start(pa_lo[:], pa_d[p, L_HI:LPAD])
                        af_hi = pool_af.tile([L_HI, 2, FEAT], BF16, tag="af_hi")
                        af_lo = pool_af.tile(
                            [LPAD - L_HI, 2, FEAT], BF16, tag="af_lo"
                        )
                        nc.sync.dma_start(af_hi[:], af_d[p, 0:L_HI])
                        nc.sync.dma_start(af_lo[:], af_d[p, L_HI:LPAD])

                        # output rows for this pair, staged flat on partition 0
                        ob = pool_ob.tile([1, 2 * FEAT], F32)

                        # -------- phase 1: scores for both batches of the pair --------
                        sc = pool_sc.tile([1, 2, L], F32)
                        for hc in range(NHC):
                            hsl = slice(hc * 128, (hc + 1) * 128)
                            e_bf = pool_e.tile([128, 2, L], BF16)
                            for jb in range(2):
                                b = b0 + jb
                                tp = pool_tp.tile([128, L], BF16)
                                t1 = nc.tensor.matmul(
                                    tp[:, 0:L_HI],
                                    pa_hi[:, jb, hsl],
                                    identity_bf[:],
                                    is_transpose=True,
                                    start=True,
                                    stop=False,
                                )
                                t2 = nc.tensor.matmul(
                                    tp[:, L_HI:L],
                                    pa_lo[0:L_LO, jb, hsl],
                                    identity_bf[:L_LO, :L_LO],
                                    is_transpose=True,
                                    start=False,
                                    stop=True,
                                )
                                add_dep_helper(t2.ins, t1.ins, sync=False, reason="tpord")
                                nc.scalar.activation(
                                    e_bf[:, jb, :], tp[:], Tanh, bias=wahT[:, hc, b : b + 1]
                                )
                            nc.tensor.matmul(
                                sc[:],
                                awT[:, hc : hc + 1],
                                e_bf[:],
                                start=(hc == 0),
                                stop=(hc == NHC - 1),
                            )

                        for jb in range(2):
                            b = b0 + jb
                            # exp with fused row-sum, then 1/sum
                            nc.scalar.activation(
                                expS[0:1, b * LP : b * LP + L],
                                sc[0:1, jb, :],
                                Exp,
                                accum_out=sums[0:1, b : b + 1],
                            )
                            nc.vector.reciprocal(
                                rsum[0:1, b : b + 1], sums[0:1, b : b + 1]
                            )

                            # alphaT columns via K=1 matmuls; rhs=1/sum normalizes
                            aT = pool_aT.tile([128, 2], F32)
                            if prev_aT_read is not None:
                                # bufs=1: this start=True reopens the bank; it must
                                # wait for the previous batch's col-1 read (regions
                                # are disjoint, so Tile tracks no dep itself)
                                pre_m1 = prev_aT_read
                            else:
                                pre_m1 = None
                            m1 = nc.tensor.matmul(
                                aT[:, 0:1],
                                expS[0:1, b * LP : b * LP + 128],
                                rsum[0:1, b : b + 1],
                                start=True,
                                stop=False,
                            )
                            m2 = nc.tensor.matmul(
                                aT[:, 1:2],
                                expS[0:1, b * LP + 128 : b * LP + 256],
                                rsum[0:1, b : b + 1],
                                start=False,
                                stop=True,
                            )
                            add_dep_helper(m2.ins, m1.ins, sync=False, reason="aTord")
                            if pre_m1 is not None:
                                add_dep_helper(m1.ins, pre_m1, sync=True, reason="aTwar")
                            c1 = nc.vector.tensor_copy(aT_sb[:, b, 0:1], aT[:, 0:1])
                            # col-0 read must wait until the accumulation group
                            # (closed by m2) is complete
                            add_dep_helper(c1.ins, m2.ins, sync=True, reason="aTgrp")
                            c2 = nc.vector.tensor_copy(
                                aT_sb[0:L_LO, b, 1:2], aT[0:L_LO, 1:2]
                            )
                            prev_aT_read = c2.ins

                            # -------- phase 2: weighted sum of att_feats --------
                            for q in range(NFQ):
                                fsl = slice(q * 512, (q + 1) * 512)
                                ao = pool_ao.tile([1, 512], F32)
                                nc.tensor.matmul(
                                    ao[:],
                                    aT_sb[:, b, 0:1],
                                    af_hi[:, jb, fsl],
                                    start=True,
                                    stop=False,
                                )
                                nc.tensor.matmul(
                                    ao[:],
                                    aT_sb[0:L_LO, b, 1:2],
                                    af_lo[0:L_LO, jb, fsl],
                                    start=False,
                                    stop=True,
                                )
                                osl = slice(
                                    jb * FEAT + q * 512, jb * FEAT + (q + 1) * 512
                                )
                                if q < 3:
                                    nc.vector.tensor_copy(ob[0:1, osl], ao[:])
                                else:
                                    nc.scalar.copy(ob[0:1, osl], ao[:])

                        nc.gpsimd.dma_start(out_d[b0 : b0 + 2, :], ob[:])

    if split:
        _split_sync(nc)
    return nc


_NC_CACHE = None


def _get_nc():
    global _NC_CACHE
    if _NC_CACHE is None:
        _NC_CACHE = build_nc()
    return _NC_CACHE


def _pack_pairs(x, width):
    """[BL, L, width] f32 -> [NPAIR, LPAD, 2, width] bf16, zero row padding."""
    import ml_dtypes

    out = np.zeros((NPAIR, LPAD, 2, width), dtype=ml_dtypes.bfloat16)
    # [BL, L, w] -> [NPAIR, 2, L, w] -> transpose to [NPAIR, L, 2, w]
    out[:, :L] = (
        x.reshape(NPAIR, 2, L, width).transpose(0, 2, 1, 3).astype(ml_dtypes.bfloat16)
    )
    return out


def _make_in_maps(h, att_feats, p_att_feats, Wah_w, alpha_w):
    h = np.ascontiguousarray(h, dtype=np.float32)
    att_feats = np.ascontiguousarray(att_feats, dtype=np.float32)
    p_att_feats = np.ascontiguousarray(p_att_feats, dtype=np.float32)
    Wah_w = np.ascontiguousarray(Wah_w, dtype=np.float32)
    alpha_w = np.ascontiguousarray(alpha_w, dtype=np.float32)
    in_maps = []
    for i in range(NCORES):
        sl = slice(i * BL, (i + 1) * BL)
        in_maps.append(
            {
                "h": np.ascontiguousarray(h[sl].T),
                "att_feats": _pack_pairs(att_feats[sl], FEAT),
                "p_att_feats": _pack_pairs(p_att_feats[sl], HID),
                "Wah_w": np.ascontiguousarray(Wah_w.T),
                "alpha_w": alpha_w,
            }
        )
    return in_maps


def run_spmd(h, att_feats, p_att_feats, Wah_w, alpha_w, trace=False):
    """Run the SPMD kernel; returns (full_output, BassKernelResults)."""
    from concourse.bass_utils import run_bass_kernel_spmd

    nc = _get_nc()
    in_maps = _make_in_maps(h, att_feats, p_att_feats, Wah_w, alpha_w)
    res = run_bass_kernel_spmd(nc, in_maps, list(range(NCORES)), trace=trace)
    out = np.concatenate([res.results[i]["out"] for i in range(NCORES)], axis=0)
    return out, res


def kernel(h, att_feats, p_att_feats, Wah_w, alpha_w):
    out, _ = run_spmd(h, att_feats, p_att_feats, Wah_w, alpha_w, trace=False)
    return out



# revision 12
# speedup vs baseline: 1.3238x; 1.3238x over previous
"""Trainium2 Bass kernel for additive-attention pooling.

Computation (per batch row b):
    Wah   = h @ Wah_w.T                         [B, HID]
    e     = tanh(Wah[:, None, :] + p_att_feats) [B, L, HID]
    s     = e @ alpha_w[0]                      [B, L]
    alpha = softmax(s, -1)                      [B, L]
    att   = sum_l alpha[b, l] * att_feats[b, l, :]   [B, FEAT]

Sharding: pure data parallel over the batch dim, 32 rows per core on 8
NeuronCores; the small Wah_w / alpha_w weights are replicated.

Per-core dataflow (v2 — whole-core block-diagonal formulation):

  p_att_feats is host-transposed to [h, l] layout so NO on-chip PE
  transposes are needed: ScalarE computes e = tanh(pa + Wah-bias) with
  the per-partition bias directly from SBUF, and TensorE contracts with
  alpha_w^T columns to get scores.

  The attention-weighted sum over l is reformulated whole-core: the
  (b, l) index pairs flatten to r = 196*b + l in [0, 6272) = 49 chunks
  of exactly 128 (no padding).  att_feats streams as [128, chunk, feat]
  tiles.  exp(scores) rows are transposed chunk-wise via K=1 matmuls
  into a block-diagonal weight tile aT_all[128, 49, 32] (column b of
  chunk q holds exp values where r belongs to batch b, else 0).  Then
  att'[0:32, f] accumulates in 4 PSUM banks over 49 matmuls of
  [K=128] x [M=32, N=512] — full-width PE work instead of M=1 matvecs.
  A 50th column of ones per chunk accumulates Z[b] = sum_l exp(s) in a
  [32, 1] PSUM tile, so softmax normalization folds into the final
  PSUM->SBUF copies as a per-partition 1/Z scale (DVE tensor_scalar).

  DMA rings: att_feats on the SP HWDGE ring (nc.sync), p_att on the ACT
  HWDGE ring (nc.scalar), setup weights + output on SWDGE (nc.gpsimd) —
  three independent FIFOs that share the 16 SDMA engines fairly.

The walrus build in this image accepts only one semaphore wait and one
update per instruction; _split_sync() post-processes the scheduled BIR
to spread Tile's multi-wait/multi-update sync info onto NoOp carriers.
"""

import os
import sys
import types

sys.path.insert(0, "/opt/trn_rl_repo")

# This image's antenv package lacks axon_hooks; provide it so
# concourse.bass_utils can import it (trace path) without crashing.
if "antenv.axon_hooks" not in sys.modules:
    _m = types.ModuleType("antenv.axon_hooks")

    def _set_hook(h):
        _m._hook = h

    def _get_hook():
        return getattr(_m, "_hook", None)

    _m.set_axon_ntff_profile_hook = _set_hook
    _m.get_axon_ntff_profile_hook = _get_hook
    sys.modules["antenv.axon_hooks"] = _m
    import antenv

    antenv.axon_hooks = _m

import numpy as np  # noqa: E402
import bass_rust  # noqa: E402
import concourse.bass as bass  # noqa: E402
import concourse.tile as tile  # noqa: E402
from concourse import mybir  # noqa: E402

F32 = mybir.dt.float32
BF16 = mybir.dt.bfloat16
PSUM = bass.MemorySpace.PSUM
Tanh = mybir.ActivationFunctionType.Tanh
Exp = mybir.ActivationFunctionType.Exp

B, L, RNN, HID, FEAT = 256, 196, 1024, 512, 2048
NCORES = 8
BL = B // NCORES  # batch rows per core (32)
NHC = HID // 128  # 4 h chunks
NRC = RNN // 128  # 8 r chunks
NFQ = FEAT // 512  # 4 psum-bank-sized f chunks
NPAIR = BL // 2  # 16
LP = 224  # l padded to 224 so every batch boundary in r-space is 32-aligned
RTOT = BL * LP  # 7168 = 56 * 128
NCH = RTOT // 128  # 56 l-chunks, whole core
GCH = 7  # chunks per att_feats DMA group
NG = NCH // GCH  # 8 groups
NPIECE = 4  # p_att DMA pieces
JPP = BL // NPIECE  # 8 batches per piece

AF_BUFS = int(os.environ.get("KERNEL_AF_BUFS", "2"))


def _legal_pieces(p0, p1):
    """Split a partition range [p0, p1) (32-aligned) into pieces a compute
    engine may address: start 0 (len<=128), 32 (<=32), 64 (<=64), 96 (<=32)."""
    pieces = []
    while p0 < p1:
        if p0 == 0:
            pieces.append((0, p1))
            break
        if p0 == 32:
            pieces.append((32, min(64, p1)))
            p0 = 64
            continue
        pieces.append((p0, p1))
        break
    return pieces


def _split_sync(nc):
    """walrus in this image encodes at most ONE semaphore wait and ONE
    semaphore update per instruction; Tile freely emits several. Move the
    extras onto single-wait/single-update NoOp carriers on the same engine
    (engine queues are strict FIFO, so a preceding NoOp's wait gates the
    instruction and a following NoOp's update fires after it completes)."""
    dma_types = {
        "InstDMACopy",
        "InstTensorLoad",
        "InstTensorSave",
        "InstDmaTransposeAnt",
        "InstTensorCopy",
    }
    for f in nc.m.functions:
        for bb in f.blocks:
            new = []
            changed = False
            for ins in bb.instructions:
                si = ins.sync_info
                if si is None:
                    new.append(ins)
                    continue
                waits = list(si.on_wait)
                updates = list(si.on_update)
                if len(waits) <= 1 and len(updates) <= 1:
                    new.append(ins)
                    continue
                changed = True
                tname = type(ins).__name__
                for j, w in enumerate(waits[:-1]):
                    nop = mybir.InstNoOp(name=f"{ins.name}_w{j}", ins=[], outs=[])
                    nop.engine = ins.engine
                    nop.sync_info = bass_rust.SyncInfo(on_wait=[w], on_update=[])
                    new.append(nop)
                keep_w = waits[-1:]
                post_u = []
                keep_u = updates
                if len(updates) > 1:
                    if tname in dma_types:
                        raise RuntimeError(
                            f"DMA instruction {ins.name} carries {len(updates)} "
                            "sem updates; cannot split without changing semantics"
                        )
                    keep_u = updates[:1]
                    post_u = updates[1:]
                ins.sync_info = bass_rust.SyncInfo(on_wait=keep_w, on_update=keep_u)
                new.append(ins)
                for j, u in enumerate(post_u):
                    nop = mybir.InstNoOp(name=f"{ins.name}_u{j}", ins=[], outs=[])
                    nop.engine = ins.engine
                    nop.sync_info = bass_rust.SyncInfo(on_wait=[], on_update=[u])
                    new.append(nop)
            if changed:
                bb.instructions = new


def build_nc(split=True):
    """Inputs arrive host-packed (see _make_in_maps):
      att_feats:   [NG, 128, GCH, FEAT] bf16, element (g, p, c, f) =
                   af[b, l, f] with r = 224*b + l = 128*(GCH*g + c) + p
                   (l in [196, 224) rows are zero padding)
      p_att_feats: [NPIECE, 128, JPP, NHC, L] bf16, element
                   (pc, p, j, hc, l) = pa[JPP*pc + j, l, 128*hc + p]
      h:      [RNN, BL] bf16 (host-transposed)
      Wah_w:  [RNN, HID] bf16 (host-transposed)
    """
    nc = bass.Bass()
    h_d = nc.declare_dram_parameter("h", [RNN, BL], BF16, isOutput=False)
    af_d = nc.declare_dram_parameter(
        "att_feats", [NG, 128, GCH, FEAT], BF16, isOutput=False
    )
    pa_d = nc.declare_dram_parameter(
        "p_att_feats", [NPIECE, 128, JPP, NHC, L], BF16, isOutput=False
    )
    ww_d = nc.declare_dram_parameter("Wah_w", [RNN, HID], BF16, isOutput=False)
    aw_d = nc.declare_dram_parameter("alpha_w", [1, HID], F32, isOutput=False)
    out_d = nc.declare_dram_parameter("out", [BL, FEAT], F32, isOutput=True)

    with tile.TileContext(nc) as tc:
        with tc.tile_pool(name="singles", bufs=1) as singles:
            wahT = singles.tile([128, NHC, BL], F32)  # WahT[h % 128, hc, b]
            awT = singles.tile([128, NHC], BF16)  # alpha_w^T chunks
            expS = singles.tile([1, RTOT], F32)  # exp(scores), r-major
            aT_all = singles.tile([128, NCH, BL], BF16)  # block-diag weights
            onesb = singles.tile([128, 1], BF16)  # ones col for Z matmuls
            ones11 = singles.tile([1, 1], F32)
            rz = singles.tile([BL, 1], F32)  # 1/Z per batch (partition-major)
            out_sb = singles.tile([BL, FEAT], F32)
            pa_t = singles.tile([128, NPIECE, JPP, NHC, L], BF16)

            nc.gpsimd.memset(aT_all[:], 0.0)
            nc.gpsimd.memset(onesb[:], 1.0)
            nc.gpsimd.memset(ones11[:], 1.0)
            # only the [196, 224) pad region per batch must be zero, but one
            # flat memset at t=0 (overlapped with setup DMAs) is simplest
            nc.vector.memset(expS[:], 0.0)

            # Streaming SBUF pools are allocated FIRST so their zones never
            # overlap the setup pool's — otherwise the first input DMAs
            # inherit released-zone deps on the whole setup computation.
            with (
                tc.tile_pool(name="af", bufs=AF_BUFS) as pool_af,
                tc.tile_pool(name="e", bufs=3) as pool_e,
            ):
                # input streams, emitted upfront: pool recycling (WAR deps)
                # paces the later groups automatically.
                af_t = []
                for g in range(NG):
                    t = pool_af.tile([128, GCH, FEAT], BF16, tag="af")
                    nc.sync.dma_start(t[:], af_d[g])
                    af_t.append(t)
                for pc in range(NPIECE):
                    nc.scalar.dma_start(pa_t[:, pc], pa_d[pc])

                # ---------------- setup: weights ----------------
                # h and Wah_w arrive host-transposed (r-major), so WahT is a
                # plain accumulated matmul with no on-chip transposes.
                with (
                    tc.tile_pool(name="setup_sb", bufs=1) as ssb,
                    tc.tile_pool(name="setup_ps", bufs=2, space=PSUM) as sps,
                    tc.tile_pool(name="setup_acc", bufs=1, space=PSUM) as sacc,
                ):
                    hT = ssb.tile([128, NRC, BL], BF16)
                    nc.gpsimd.dma_start(
                        hT[:], h_d[:].rearrange("(rc p) b -> p rc b", p=128)
                    )
                    wwT = ssb.tile([128, NRC, HID], BF16)
                    nc.gpsimd.dma_start(
                        wwT[:], ww_d[:].rearrange("(rc p) c -> p rc c", p=128)
                    )
                    aw_sb = ssb.tile([1, HID], F32)
                    nc.gpsimd.dma_start(aw_sb[:], aw_d[:])

                    # alpha_w^T columns (bf16 to match bf16 e tiles)
                    for hc in range(NHC):
                        ps = sps.tile([128, 1], F32, tag="aw")
                        nc.tensor.matmul(
                            ps[:],
                            aw_sb[0:1, hc * 128 : (hc + 1) * 128],
                            ones11[:],
                            start=True,
                            stop=True,
                        )
                        nc.vector.tensor_copy(awT[:, hc : hc + 1], ps[:])

                    # WahT[h, b] = sum_r Wah_w[h, r] * h[b, r]
                    wahT_ps = [
                        sacc.tile([128, BL], F32, tag=f"acc{hc}", name=f"wahT_ps{hc}")
                        for hc in range(NHC)
                    ]
                    for rc in range(NRC):
                        for hc in range(NHC):
                            nc.tensor.matmul(
                                wahT_ps[hc][:],
                                wwT[:, rc, hc * 128 : (hc + 1) * 128],
                                hT[:, rc, :],
                                start=(rc == 0),
                                stop=(rc == NRC - 1),
                            )
                    for hc in range(NHC):
                        nc.vector.tensor_copy(wahT[:, hc, :], wahT_ps[hc][:])

                # ---------------- streaming loop ----------------
                with (
                    tc.tile_pool(name="sc_ps", bufs=1, space=PSUM) as pool_sc,
                    tc.tile_pool(name="aT_ps", bufs=2, space=PSUM) as pool_aT,
                    tc.tile_pool(name="acc_ps", bufs=1, space=PSUM) as pool_acc,
                ):
                    acc = [
                        pool_acc.tile([BL, 512], F32, tag=f"acc{f}", name=f"acc{f}")
                        for f in range(NFQ)
                    ]
                    zps = pool_acc.tile([BL, 1], F32, tag="z", name="zps")

                    # chunk q's alpha values are complete after pair rdy[q]
                    ready = [[] for _ in range(NPAIR)]
                    for q in range(NCH):
                        rb = (128 * q + 127) // LP
                        ready[rb // 2].append(q)

                    for pr in range(NPAIR):
                        # -------- phase 1: scores for the pair --------
                        sc = pool_sc.tile([1, 2, L], F32, tag="sc")
                        for hc in range(NHC):
                            e_bf = pool_e.tile([128, 2, L], BF16, tag=f"e{hc}")
                            for jb in range(2):
                                b = 2 * pr + jb
                                pc, j = divmod(b, JPP)
                                nc.scalar.activation(
                                    e_bf[:, jb, :],
                                    pa_t[:, pc, j, hc, :],
                                    Tanh,
                                    bias=wahT[:, hc, b : b + 1],
                                )
                            nc.tensor.matmul(
                                sc[:],
                                awT[:, hc : hc + 1],
                                e_bf[:],
                                start=(hc == 0),
                                stop=(hc == NHC - 1),
                            )
                        for jb in range(2):
                            b = 2 * pr + jb
                            nc.scalar.activation(
                                expS[0:1, b * LP : b * LP + L], sc[0:1, jb, :], Exp
                            )

                        # -------- phase 2 for chunks completed by this pair --------
                        for q in ready[pr]:
                            aT = pool_aT.tile([128, 1], F32, tag="aT")
                            nc.tensor.matmul(
                                aT[:],
                                expS[0:1, 128 * q : 128 * q + 128],
                                ones11[:],
                                start=True,
                                stop=True,
                            )
                            # scatter into the block-diagonal weight column(s)
                            r0 = 128 * q
                            r = r0
                            while r < r0 + 128:
                                b = r // LP
                                seg_end = min(r0 + 128, (b + 1) * LP)
                                for p0, p1 in _legal_pieces(r - r0, seg_end - r0):
                                    nc.vector.tensor_copy(
                                        aT_all[p0:p1, q, b : b + 1],
                                        aT[p0:p1, 0:1],
                                    )
                                r = seg_end
                            g, qq = divmod(q, GCH)
                            lhs = aT_all[:, q, :]
                            for f in range(NFQ):
                                nc.tensor.matmul(
                                    acc[f][:],
                                    lhs,
                                    af_t[g][:, qq, f * 512 : (f + 1) * 512],
                                    start=(q == 0),
                                    stop=(q == NCH - 1),
                                )
                            nc.tensor.matmul(
                                zps[:],
                                lhs,
                                onesb[:],
                                start=(q == 0),
                                stop=(q == NCH - 1),
                            )

                    # -------- normalize + store --------
                    nc.vector.reciprocal(rz[:], zps[:])
                    for f in range(NFQ):
                        nc.vector.tensor_scalar_mul(
                            out_sb[:, f * 512 : (f + 1) * 512], acc[f][:], rz[:]
                        )
                    nc.gpsimd.dma_start(out_d[:], out_sb[:])

    if split:
        _split_sync(nc)
    return nc


_NC_CACHE = None


def _get_nc():
    global _NC_CACHE
    if _NC_CACHE is None:
        _NC_CACHE = build_nc()
    return _NC_CACHE


def _make_in_maps(h, att_feats, p_att_feats, Wah_w, alpha_w):
    import ml_dtypes

    bf = ml_dtypes.bfloat16
    h = np.ascontiguousarray(h, dtype=np.float32)
    att_feats = np.ascontiguousarray(att_feats, dtype=np.float32)
    p_att_feats = np.ascontiguousarray(p_att_feats, dtype=np.float32)
    Wah_w = np.ascontiguousarray(Wah_w, dtype=np.float32)
    alpha_w = np.ascontiguousarray(alpha_w, dtype=np.float32)
    wwT = np.ascontiguousarray(Wah_w.T.astype(bf))
    in_maps = []
    for i in range(NCORES):
        sl = slice(i * BL, (i + 1) * BL)
        # att_feats -> r-major chunks (l padded to LP): [NG, 128, GCH, FEAT]
        af_pad = np.zeros((BL, LP, FEAT), dtype=bf)
        af_pad[:, :L] = att_feats[sl]
        af = af_pad.reshape(NG, GCH, 128, FEAT).transpose(0, 2, 1, 3)
        # p_att -> [NPIECE, 128, JPP, NHC, L] (h-major on partitions)
        pa = (
            p_att_feats[sl]
            .reshape(NPIECE, JPP, L, NHC, 128)
            .transpose(0, 4, 1, 3, 2)
            .astype(bf)
        )
        in_maps.append(
            {
                "h": np.ascontiguousarray(h[sl].T.astype(bf)),
                "att_feats": np.ascontiguousarray(af),
                "p_att_feats": np.ascontiguousarray(pa),
                "Wah_w": wwT,
                "alpha_w": alpha_w,
            }
        )
    return in_maps


def run_spmd(h, att_feats, p_att_feats, Wah_w, alpha_w, trace=False):
    """Run the SPMD kernel; returns (full_output, BassKernelResults)."""
    from concourse.bass_utils import run_bass_kernel_spmd

    nc = _get_nc()
    in_maps = _make_in_maps(h, att_feats, p_att_feats, Wah_w, alpha_w)
    res = run_bass_kernel_spmd(nc, in_maps, list(range(NCORES)), trace=trace)
    out = np.concatenate([res.results[i]["out"] for i in range(NCORES)], axis=0)
    return out, res


def kernel(h, att_feats, p_att_feats, Wah_w, alpha_w):
    out, _ = run_spmd(h, att_feats, p_att_feats, Wah_w, alpha_w, trace=False)
    return out


# revision 19
# speedup vs baseline: 1.4541x; 1.0984x over previous
"""Trainium2 Bass kernel for additive-attention pooling.

Computation (per batch row b):
    Wah   = h @ Wah_w.T                         [B, HID]
    e     = tanh(Wah[:, None, :] + p_att_feats) [B, L, HID]
    s     = e @ alpha_w[0]                      [B, L]
    alpha = softmax(s, -1)                      [B, L]
    att   = sum_l alpha[b, l] * att_feats[b, l, :]   [B, FEAT]

Sharding: pure data parallel over the batch dim, 32 rows per core on 8
NeuronCores; the small Wah_w / alpha_w weights are replicated.

Per-core dataflow (v2 — whole-core block-diagonal formulation):

  p_att_feats is host-transposed to [h, l] layout so NO on-chip PE
  transposes are needed: ScalarE computes e = tanh(pa + Wah-bias) with
  the per-partition bias directly from SBUF, and TensorE contracts with
  alpha_w^T columns to get scores.

  The attention-weighted sum over l is reformulated whole-core: the
  (b, l) index pairs flatten to r = 196*b + l in [0, 6272) = 49 chunks
  of exactly 128 (no padding).  att_feats streams as [128, chunk, feat]
  tiles.  exp(scores) rows are transposed chunk-wise via K=1 matmuls
  into a block-diagonal weight tile aT_all[128, 49, 32] (column b of
  chunk q holds exp values where r belongs to batch b, else 0).  Then
  att'[0:32, f] accumulates in 4 PSUM banks over 49 matmuls of
  [K=128] x [M=32, N=512] — full-width PE work instead of M=1 matvecs.
  A 50th column of ones per chunk accumulates Z[b] = sum_l exp(s) in a
  [32, 1] PSUM tile, so softmax normalization folds into the final
  PSUM->SBUF copies as a per-partition 1/Z scale (DVE tensor_scalar).

  DMA rings: att_feats on the SP HWDGE ring (nc.sync), p_att on the ACT
  HWDGE ring (nc.scalar), setup weights + output on SWDGE (nc.gpsimd) —
  three independent FIFOs that share the 16 SDMA engines fairly.

The walrus build in this image accepts only one semaphore wait and one
update per instruction; _split_sync() post-processes the scheduled BIR
to spread Tile's multi-wait/multi-update sync info onto NoOp carriers.
"""

import os
import sys
import types

sys.path.insert(0, "/opt/trn_rl_repo")

# This image's antenv package lacks axon_hooks; provide it so
# concourse.bass_utils can import it (trace path) without crashing.
if "antenv.axon_hooks" not in sys.modules:
    _m = types.ModuleType("antenv.axon_hooks")

    def _set_hook(h):
        _m._hook = h

    def _get_hook():
        return getattr(_m, "_hook", None)

    _m.set_axon_ntff_profile_hook = _set_hook
    _m.get_axon_ntff_profile_hook = _get_hook
    sys.modules["antenv.axon_hooks"] = _m
    import antenv

    antenv.axon_hooks = _m

import numpy as np  # noqa: E402
import bass_rust  # noqa: E402
import concourse.bass as bass  # noqa: E402
import concourse.tile as tile  # noqa: E402
from concourse import mybir  # noqa: E402

F32 = mybir.dt.float32
BF16 = mybir.dt.bfloat16
PSUM = bass.MemorySpace.PSUM
Tanh = mybir.ActivationFunctionType.Tanh
Exp = mybir.ActivationFunctionType.Exp

B, L, RNN, HID, FEAT = 256, 196, 1024, 512, 2048
NCORES = 8
BL = B // NCORES  # batch rows per core (32)
NHC = HID // 128  # 4 h chunks
NRC = RNN // 128  # 8 r chunks
NFQ = FEAT // 512  # 4 psum-bank-sized f chunks
NPAIR = BL // 2  # 16
LP = 224  # l padded to 224 so every batch boundary in r-space is 32-aligned
RTOT = BL * LP  # 7168 = 56 * 128
NCH = RTOT // 128  # 56 l-chunks, whole core
GCH = 7  # chunks per att_feats DMA group
NG = NCH // GCH  # 8 groups
NPIECE = 4  # p_att DMA pieces
JPP = BL // NPIECE  # 8 batches per piece

AF_BUFS = int(os.environ.get("KERNEL_AF_BUFS", "2"))


def _legal_pieces(p0, p1):
    """Split a partition range [p0, p1) (32-aligned) into pieces a compute
    engine may address: start 0 (len<=128), 32 (<=32), 64 (<=64), 96 (<=32)."""
    pieces = []
    while p0 < p1:
        if p0 == 0:
            pieces.append((0, p1))
            break
        if p0 == 32:
            pieces.append((32, min(64, p1)))
            p0 = 64
            continue
        pieces.append((p0, p1))
        break
    return pieces


def _split_sync(nc):
    """walrus in this image encodes at most ONE semaphore wait and ONE
    semaphore update per instruction; Tile freely emits several. Move the
    extras onto single-wait/single-update NoOp carriers on the same engine
    (engine queues are strict FIFO, so a preceding NoOp's wait gates the
    instruction and a following NoOp's update fires after it completes)."""
    dma_types = {
        "InstDMACopy",
        "InstTensorLoad",
        "InstTensorSave",
        "InstDmaTransposeAnt",
        "InstTensorCopy",
    }
    for f in nc.m.functions:
        for bb in f.blocks:
            new = []
            changed = False
            for ins in bb.instructions:
                si = ins.sync_info
                if si is None:
                    new.append(ins)
                    continue
                waits = list(si.on_wait)
                updates = list(si.on_update)
                if len(waits) <= 1 and len(updates) <= 1:
                    new.append(ins)
                    continue
                changed = True
                tname = type(ins).__name__
                for j, w in enumerate(waits[:-1]):
                    nop = mybir.InstNoOp(name=f"{ins.name}_w{j}", ins=[], outs=[])
                    nop.engine = ins.engine
                    nop.sync_info = bass_rust.SyncInfo(on_wait=[w], on_update=[])
                    new.append(nop)
                keep_w = waits[-1:]
                post_u = []
                keep_u = updates
                if len(updates) > 1:
                    if tname in dma_types:
                        raise RuntimeError(
                            f"DMA instruction {ins.name} carries {len(updates)} "
                            "sem updates; cannot split without changing semantics"
                        )
                    keep_u = updates[:1]
                    post_u = updates[1:]
                ins.sync_info = bass_rust.SyncInfo(on_wait=keep_w, on_update=keep_u)
                new.append(ins)
                for j, u in enumerate(post_u):
                    nop = mybir.InstNoOp(name=f"{ins.name}_u{j}", ins=[], outs=[])
                    nop.engine = ins.engine
                    nop.sync_info = bass_rust.SyncInfo(on_wait=[], on_update=[u])
                    new.append(nop)
            if changed:
                bb.instructions = new


def build_nc(split=True):
    """Inputs arrive host-packed (see _make_in_maps):
      att_feats:   [NG, 128, GCH, FEAT] bf16, element (g, p, c, f) =
                   af[b, l, f] with r = 224*b + l = 128*(GCH*g + c) + p
                   (l in [196, 224) rows are zero padding)
      p_att_feats: [NPIECE, 128, JPP, NHC, L] bf16, element
                   (pc, p, j, hc, l) = pa[JPP*pc + j, l, 128*hc + p]
      h:      [RNN, BL] bf16 (host-transposed)
      Wah_w:  [RNN, HID] bf16 (host-transposed)
    """
    nc = bass.Bass()
    h_d = nc.declare_dram_parameter("h", [128, NRC, BL], BF16, isOutput=False)
    af_d = nc.declare_dram_parameter(
        "att_feats", [NG, 128, GCH, FEAT], BF16, isOutput=False
    )
    pa_d = nc.declare_dram_parameter(
        "p_att_feats", [NPIECE, 128, JPP, NHC, L], BF16, isOutput=False
    )
    ww_d = nc.declare_dram_parameter("Wah_w", [128, NRC, HID], BF16, isOutput=False)
    aw_d = nc.declare_dram_parameter("alpha_w", [1, HID], F32, isOutput=False)
    out_d = nc.declare_dram_parameter("out", [BL, FEAT], F32, isOutput=True)

    with tile.TileContext(nc) as tc:
        with tc.tile_pool(name="singles", bufs=1) as singles:
            wahT = singles.tile([128, NHC, BL], F32)  # WahT[h % 128, hc, b]
            awT = singles.tile([128, NHC], BF16)  # alpha_w^T chunks
            expS = singles.tile([1, RTOT], F32)  # exp(scores), r-major
            aT_all = singles.tile([128, NCH, BL], BF16)  # block-diag weights
            onesb = singles.tile([128, 1], BF16)  # ones col for Z matmuls
            ones11 = singles.tile([1, 1], F32)
            rz = singles.tile([BL, 1], F32)  # 1/Z per batch (partition-major)
            out_sb = singles.tile([BL, FEAT], F32)
            pa_t = singles.tile([128, NPIECE, JPP, NHC, L], BF16)

            nc.gpsimd.memset(aT_all[:], 0.0)
            nc.gpsimd.memset(onesb[:], 1.0)
            nc.gpsimd.memset(ones11[:], 1.0)
            # expS pad columns are never copied into aT_all (scatter clamps
            # to real rows), but the chunk transposes read them: zero once.
            nc.gpsimd.memset(expS[:], 0.0)

            # Streaming SBUF pools are allocated FIRST so their zones never
            # overlap the setup pool's — otherwise the first input DMAs
            # inherit released-zone deps on the whole setup computation.
            with (
                tc.tile_pool(name="af", bufs=AF_BUFS) as pool_af,
                tc.tile_pool(name="e", bufs=3) as pool_e,
            ):
                # ---------------- setup: weights ----------------
                # h and Wah_w arrive host-packed in the exact SBUF layout, as
                # the FIRST transfers on the two HWDGE rings so phase 1 can
                # start immediately; the big streams queue up behind them.
                with (
                    tc.tile_pool(name="setup_sb", bufs=1) as ssb,
                    tc.tile_pool(name="setup_ps", bufs=2, space=PSUM) as sps,
                    tc.tile_pool(name="setup_acc", bufs=1, space=PSUM) as sacc,
                ):
                    hT = ssb.tile([128, NRC, BL], BF16)
                    nc.sync.dma_start(hT[:], h_d[:])
                    wwT = ssb.tile([128, NRC, HID], BF16)
                    nc.sync.dma_start(wwT[:], ww_d[:])
                    aw_sb = ssb.tile([1, HID], F32)
                    nc.scalar.dma_start(aw_sb[:], aw_d[:])

                    # input streams, emitted upfront: pool recycling (WAR
                    # deps) paces the later att_feats groups automatically.
                    af_t = []
                    for g in range(NG):
                        t = pool_af.tile([128, GCH, FEAT], BF16, tag="af")
                        nc.sync.dma_start(t[:], af_d[g])
                        af_t.append(t)
                    for pc in range(NPIECE):
                        nc.scalar.dma_start(pa_t[:, pc], pa_d[pc])

                    # alpha_w^T columns (bf16 to match bf16 e tiles)
                    for hc in range(NHC):
                        ps = sps.tile([128, 1], F32, tag="aw")
                        nc.tensor.matmul(
                            ps[:],
                            aw_sb[0:1, hc * 128 : (hc + 1) * 128],
                            ones11[:],
                            start=True,
                            stop=True,
                        )
                        nc.vector.tensor_copy(awT[:, hc : hc + 1], ps[:])

                    # WahT[h, b] = sum_r Wah_w[h, r] * h[b, r]
                    wahT_ps = [
                        sacc.tile([128, BL], F32, tag=f"acc{hc}", name=f"wahT_ps{hc}")
                        for hc in range(NHC)
                    ]
                    for rc in range(NRC):
                        for hc in range(NHC):
                            nc.tensor.matmul(
                                wahT_ps[hc][:],
                                wwT[:, rc, hc * 128 : (hc + 1) * 128],
                                hT[:, rc, :],
                                start=(rc == 0),
                                stop=(rc == NRC - 1),
                            )
                    for hc in range(NHC):
                        nc.vector.tensor_copy(wahT[:, hc, :], wahT_ps[hc][:])

                # ---------------- streaming loop ----------------
                with (
                    tc.tile_pool(name="sc_ps", bufs=1, space=PSUM) as pool_sc,
                    tc.tile_pool(name="aT_ps", bufs=2, space=PSUM) as pool_aT,
                    tc.tile_pool(name="acc_ps", bufs=1, space=PSUM) as pool_acc,
                ):
                    acc = [
                        pool_acc.tile([BL, 512], F32, tag=f"acc{f}", name=f"acc{f}")
                        for f in range(NFQ)
                    ]
                    zps = pool_acc.tile([BL, 1], F32, tag="z", name="zps")

                    # chunk q's alpha values are complete after pair rdy[q]
                    ready = [[] for _ in range(NPAIR)]
                    for q in range(NCH):
                        rb = (128 * q + 127) // LP
                        ready[rb // 2].append(q)

                    for pr in range(NPAIR):
                        # -------- phase 1: scores for the pair --------
                        sc = pool_sc.tile([1, 2, L], F32, tag="sc")
                        for hc in range(NHC):
                            e_bf = pool_e.tile([128, 2, L], BF16, tag=f"e{hc}")
                            for jb in range(2):
                                b = 2 * pr + jb
                                pc, j = divmod(b, JPP)
                                nc.scalar.activation(
                                    e_bf[:, jb, :],
                                    pa_t[:, pc, j, hc, :],
                                    Tanh,
                                    bias=wahT[:, hc, b : b + 1],
                                )
                            nc.tensor.matmul(
                                sc[:],
                                awT[:, hc : hc + 1],
                                e_bf[:],
                                start=(hc == 0),
                                stop=(hc == NHC - 1),
                            )
                        for jb in range(2):
                            b = 2 * pr + jb
                            nc.scalar.activation(
                                expS[0:1, b * LP : b * LP + L], sc[0:1, jb, :], Exp
                            )

                        # -------- phase 2 for chunks completed by this pair --------
                        for q in ready[pr]:
                            aT = pool_aT.tile([128, 1], F32, tag="aT")
                            nc.tensor.matmul(
                                aT[:],
                                expS[0:1, 128 * q : 128 * q + 128],
                                ones11[:],
                                start=True,
                                stop=True,
                            )
                            # scatter into the block-diagonal weight column(s);
                            # clamp to real rows so [196, 224) pad garbage is
                            # never copied (aT_all pad rows stay memset-zero)
                            r0 = 128 * q
                            r = r0
                            while r < r0 + 128:
                                b = r // LP
                                seg_end = min(r0 + 128, b * LP + L)
                                for p0, p1 in _legal_pieces(
                                    r - r0, max(seg_end, r) - r0
                                ):
                                    nc.vector.tensor_copy(
                                        aT_all[p0:p1, q, b : b + 1],
                                        aT[p0:p1, 0:1],
                                    )
                                r = (b + 1) * LP
                            g, qq = divmod(q, GCH)
                            lhs = aT_all[:, q, :]
                            for f in range(NFQ):
                                nc.tensor.matmul(
                                    acc[f][:],
                                    lhs,
                                    af_t[g][:, qq, f * 512 : (f + 1) * 512],
                                    start=(q == 0),
                                    stop=(q == NCH - 1),
                                )
                            nc.tensor.matmul(
                                zps[:],
                                lhs,
                                onesb[:],
                                start=(q == 0),
                                stop=(q == NCH - 1),
                            )

                    # -------- normalize + store --------
                    nc.vector.reciprocal(rz[:], zps[:])
                    for f in range(NFQ):
                        nc.vector.tensor_scalar_mul(
                            out_sb[:, f * 512 : (f + 1) * 512], acc[f][:], rz[:]
                        )
                    nc.gpsimd.dma_start(out_d[:], out_sb[:])

    if split:
        _split_sync(nc)
    return nc


_NC_CACHE = None


def _get_nc():
    global _NC_CACHE
    if _NC_CACHE is None:
        _NC_CACHE = build_nc()
    return _NC_CACHE


def _make_in_maps(h, att_feats, p_att_feats, Wah_w, alpha_w):
    import ml_dtypes

    bf = ml_dtypes.bfloat16
    h = np.ascontiguousarray(h, dtype=np.float32)
    att_feats = np.ascontiguousarray(att_feats, dtype=np.float32)
    p_att_feats = np.ascontiguousarray(p_att_feats, dtype=np.float32)
    Wah_w = np.ascontiguousarray(Wah_w, dtype=np.float32)
    alpha_w = np.ascontiguousarray(alpha_w, dtype=np.float32)
    # Wah_w [HID, RNN] -> [128, NRC, HID]: element (p, rc, c) = W[c, 128*rc+p]
    wwT = np.ascontiguousarray(
        Wah_w.T.reshape(NRC, 128, HID).transpose(1, 0, 2).astype(bf)
    )
    in_maps = []
    for i in range(NCORES):
        sl = slice(i * BL, (i + 1) * BL)
        # att_feats -> r-major chunks (l padded to LP): [NG, 128, GCH, FEAT]
        af_pad = np.zeros((BL, LP, FEAT), dtype=bf)
        af_pad[:, :L] = att_feats[sl]
        af = af_pad.reshape(NG, GCH, 128, FEAT).transpose(0, 2, 1, 3)
        # p_att -> [NPIECE, 128, JPP, NHC, L] (h-major on partitions)
        pa = (
            p_att_feats[sl]
            .reshape(NPIECE, JPP, L, NHC, 128)
            .transpose(0, 4, 1, 3, 2)
            .astype(bf)
        )
        # h [BL, RNN] -> [128, NRC, BL]: element (p, rc, b) = h[b, 128*rc+p]
        hT = h[sl].T.reshape(NRC, 128, BL).transpose(1, 0, 2).astype(bf)
        in_maps.append(
            {
                "h": np.ascontiguousarray(hT),
                "att_feats": np.ascontiguousarray(af),
                "p_att_feats": np.ascontiguousarray(pa),
                "Wah_w": wwT,
                "alpha_w": alpha_w,
            }
        )
    return in_maps


def run_spmd(h, att_feats, p_att_feats, Wah_w, alpha_w, trace=False):
    """Run the SPMD kernel; returns (full_output, BassKernelResults)."""
    from concourse.bass_utils import run_bass_kernel_spmd

    nc = _get_nc()
    in_maps = _make_in_maps(h, att_feats, p_att_feats, Wah_w, alpha_w)
    res = run_bass_kernel_spmd(nc, in_maps, list(range(NCORES)), trace=trace)
    out = np.concatenate([res.results[i]["out"] for i in range(NCORES)], axis=0)
    return out, res


def kernel(h, att_feats, p_att_feats, Wah_w, alpha_w):
    out, _ = run_spmd(h, att_feats, p_att_feats, Wah_w, alpha_w, trace=False)
    return out


# revision 21
# speedup vs baseline: 1.6264x; 1.1185x over previous
"""Trainium2 Bass kernel for additive-attention pooling.

Computation (per batch row b):
    Wah   = h @ Wah_w.T                         [B, HID]
    e     = tanh(Wah[:, None, :] + p_att_feats) [B, L, HID]
    s     = e @ alpha_w[0]                      [B, L]
    alpha = softmax(s, -1)                      [B, L]
    att   = sum_l alpha[b, l] * att_feats[b, l, :]   [B, FEAT]

Sharding: pure data parallel over the batch dim, 32 rows per core on 8
NeuronCores; the small Wah_w / alpha_w weights are replicated.

Per-core dataflow (v2 — whole-core block-diagonal formulation):

  p_att_feats is host-transposed to [h, l] layout so NO on-chip PE
  transposes are needed: ScalarE computes e = tanh(pa + Wah-bias) with
  the per-partition bias directly from SBUF, and TensorE contracts with
  alpha_w^T columns to get scores.

  The attention-weighted sum over l is reformulated whole-core: the
  (b, l) index pairs flatten to r = 196*b + l in [0, 6272) = 49 chunks
  of exactly 128 (no padding).  att_feats streams as [128, chunk, feat]
  tiles.  exp(scores) rows are transposed chunk-wise via K=1 matmuls
  into a block-diagonal weight tile aT_all[128, 49, 32] (column b of
  chunk q holds exp values where r belongs to batch b, else 0).  Then
  att'[0:32, f] accumulates in 4 PSUM banks over 49 matmuls of
  [K=128] x [M=32, N=512] — full-width PE work instead of M=1 matvecs.
  A 50th column of ones per chunk accumulates Z[b] = sum_l exp(s) in a
  [32, 1] PSUM tile, so softmax normalization folds into the final
  PSUM->SBUF copies as a per-partition 1/Z scale (DVE tensor_scalar).

  DMA rings: att_feats on the SP HWDGE ring (nc.sync), p_att on the ACT
  HWDGE ring (nc.scalar), setup weights + output on SWDGE (nc.gpsimd) —
  three independent FIFOs that share the 16 SDMA engines fairly.

The walrus build in this image accepts only one semaphore wait and one
update per instruction; _split_sync() post-processes the scheduled BIR
to spread Tile's multi-wait/multi-update sync info onto NoOp carriers.
"""

import os
import sys
import types

sys.path.insert(0, "/opt/trn_rl_repo")

# This image's antenv package lacks axon_hooks; provide it so
# concourse.bass_utils can import it (trace path) without crashing.
if "antenv.axon_hooks" not in sys.modules:
    _m = types.ModuleType("antenv.axon_hooks")

    def _set_hook(h):
        _m._hook = h

    def _get_hook():
        return getattr(_m, "_hook", None)

    _m.set_axon_ntff_profile_hook = _set_hook
    _m.get_axon_ntff_profile_hook = _get_hook
    sys.modules["antenv.axon_hooks"] = _m
    import antenv

    antenv.axon_hooks = _m

import numpy as np  # noqa: E402
import bass_rust  # noqa: E402
import concourse.bass as bass  # noqa: E402
import concourse.tile as tile  # noqa: E402
from concourse import mybir  # noqa: E402

F32 = mybir.dt.float32
BF16 = mybir.dt.bfloat16
PSUM = bass.MemorySpace.PSUM
Tanh = mybir.ActivationFunctionType.Tanh
Exp = mybir.ActivationFunctionType.Exp

B, L, RNN, HID, FEAT = 256, 196, 1024, 512, 2048
NCORES = 8
BL = B // NCORES  # batch rows per core (32)
NHC = HID // 128  # 4 h chunks
NRC = RNN // 128  # 8 r chunks
NFQ = FEAT // 512  # 4 psum-bank-sized f chunks
NPAIR = BL // 2  # 16
LP = 224  # l padded to 224 so every batch boundary in r-space is 32-aligned
RTOT = BL * LP  # 7168 = 56 * 128
NCH = RTOT // 128  # 56 l-chunks, whole core
GCH = 7  # chunks per att_feats DMA group
NG = NCH // GCH  # 8 groups
NPIECE = 4  # p_att DMA pieces
JPP = BL // NPIECE  # 8 batches per piece

AF_BUFS = int(os.environ.get("KERNEL_AF_BUFS", "2"))


def _legal_pieces(p0, p1):
    """Split a partition range [p0, p1) (32-aligned) into pieces a compute
    engine may address: start 0 (len<=128), 32 (<=32), 64 (<=64), 96 (<=32)."""
    pieces = []
    while p0 < p1:
        if p0 == 0:
            pieces.append((0, p1))
            break
        if p0 == 32:
            pieces.append((32, min(64, p1)))
            p0 = 64
            continue
        pieces.append((p0, p1))
        break
    return pieces


def _split_sync(nc):
    """walrus in this image encodes at most ONE semaphore wait and ONE
    semaphore update per instruction; Tile freely emits several. Move the
    extras onto single-wait/single-update NoOp carriers on the same engine
    (engine queues are strict FIFO, so a preceding NoOp's wait gates the
    instruction and a following NoOp's update fires after it completes)."""
    dma_types = {
        "InstDMACopy",
        "InstTensorLoad",
        "InstTensorSave",
        "InstDmaTransposeAnt",
        "InstTensorCopy",
    }
    for f in nc.m.functions:
        for bb in f.blocks:
            new = []
            changed = False
            for ins in bb.instructions:
                si = ins.sync_info
                if si is None:
                    new.append(ins)
                    continue
                waits = list(si.on_wait)
                updates = list(si.on_update)
                if len(waits) <= 1 and len(updates) <= 1:
                    new.append(ins)
                    continue
                changed = True
                tname = type(ins).__name__
                for j, w in enumerate(waits[:-1]):
                    nop = mybir.InstNoOp(name=f"{ins.name}_w{j}", ins=[], outs=[])
                    nop.engine = ins.engine
                    nop.sync_info = bass_rust.SyncInfo(on_wait=[w], on_update=[])
                    new.append(nop)
                keep_w = waits[-1:]
                post_u = []
                keep_u = updates
                if len(updates) > 1:
                    if tname in dma_types:
                        raise RuntimeError(
                            f"DMA instruction {ins.name} carries {len(updates)} "
                            "sem updates; cannot split without changing semantics"
                        )
                    keep_u = updates[:1]
                    post_u = updates[1:]
                ins.sync_info = bass_rust.SyncInfo(on_wait=keep_w, on_update=keep_u)
                new.append(ins)
                for j, u in enumerate(post_u):
                    nop = mybir.InstNoOp(name=f"{ins.name}_u{j}", ins=[], outs=[])
                    nop.engine = ins.engine
                    nop.sync_info = bass_rust.SyncInfo(on_wait=[], on_update=[u])
                    new.append(nop)
            if changed:
                bb.instructions = new


def build_nc(split=True):
    """Inputs arrive host-packed (see _make_in_maps):
      att_feats:   [NG, 128, GCH, FEAT] bf16, element (g, p, c, f) =
                   af[b, l, f] with r = 224*b + l = 128*(GCH*g + c) + p
                   (l in [196, 224) rows are zero padding)
      p_att_feats: [NPIECE, 128, JPP, NHC, L] bf16, element
                   (pc, p, j, hc, l) = pa[JPP*pc + j, l, 128*hc + p]
      h:      [RNN, BL] bf16 (host-transposed)
      Wah_w:  [RNN, HID] bf16 (host-transposed)
    """
    nc = bass.Bass()
    h_d = nc.declare_dram_parameter("h", [128, NRC, BL], BF16, isOutput=False)
    af_d = nc.declare_dram_parameter(
        "att_feats", [NG, 128, GCH, FEAT], BF16, isOutput=False
    )
    pa_d = nc.declare_dram_parameter(
        "p_att_feats", [NPIECE, 128, JPP, NHC, L], BF16, isOutput=False
    )
    ww_d = nc.declare_dram_parameter("Wah_w", [128, NRC, HID], BF16, isOutput=False)
    aw_d = nc.declare_dram_parameter("alpha_w", [1, HID], F32, isOutput=False)
    out_d = nc.declare_dram_parameter("out", [BL, FEAT], F32, isOutput=True)

    with tile.TileContext(nc) as tc:
        with tc.tile_pool(name="singles", bufs=1) as singles:
            wahT = singles.tile([128, NHC, BL], F32)  # WahT[h % 128, hc, b]
            awT = singles.tile([128, NHC], BF16)  # alpha_w^T chunks
            expS = singles.tile([1, RTOT], F32)  # exp(scores), r-major
            aT_all = singles.tile([128, NCH, BL], BF16)  # block-diag weights
            onesb = singles.tile([128, 1], BF16)  # ones col for Z matmuls
            ones11 = singles.tile([1, 1], F32)
            rz = singles.tile([BL, 1], F32)  # 1/Z per batch (partition-major)
            out_sb = singles.tile([BL, FEAT], F32)
            pa_t = singles.tile([128, NPIECE, JPP, NHC, L], BF16)

            nc.gpsimd.memset(aT_all[:], 0.0)
            nc.gpsimd.memset(onesb[:], 1.0)
            nc.gpsimd.memset(ones11[:], 1.0)
            # expS pad columns are never copied into aT_all (scatter clamps
            # to real rows), but the chunk transposes read them: zero once.
            nc.gpsimd.memset(expS[:], 0.0)

            # Streaming SBUF pools are allocated FIRST so their zones never
            # overlap the setup pool's — otherwise the first input DMAs
            # inherit released-zone deps on the whole setup computation.
            with (
                tc.tile_pool(name="af", bufs=AF_BUFS) as pool_af,
                tc.tile_pool(name="e", bufs=3) as pool_e,
            ):
                # ---------------- setup: weights ----------------
                # h and Wah_w arrive host-packed in the exact SBUF layout, as
                # the FIRST transfers on the two HWDGE rings so phase 1 can
                # start immediately; the big streams queue up behind them.
                with (
                    tc.tile_pool(name="setup_sb", bufs=1) as ssb,
                    tc.tile_pool(name="setup_ps", bufs=2, space=PSUM) as sps,
                    tc.tile_pool(name="setup_acc", bufs=1, space=PSUM) as sacc,
                ):
                    hT = ssb.tile([128, NRC, BL], BF16)
                    nc.sync.dma_start(hT[:], h_d[:])
                    wwT = ssb.tile([128, NRC, HID], BF16)
                    nc.sync.dma_start(wwT[:], ww_d[:])
                    aw_sb = ssb.tile([1, HID], F32)
                    nc.scalar.dma_start(aw_sb[:], aw_d[:])

                    # input streams, all on the SP ring (strict FIFO): pa
                    # pieces interleaved ahead of af groups so phase 1 is
                    # never input-starved; pool recycling (WAR deps) paces
                    # the later att_feats groups automatically.
                    af_t = []

                    def emit_af(g):
                        t = pool_af.tile([128, GCH, FEAT], BF16, tag="af")
                        nc.sync.dma_start(t[:], af_d[g])
                        af_t.append(t)

                    for pc in range(NPIECE):
                        nc.sync.dma_start(pa_t[:, pc], pa_d[pc])
                        emit_af(pc)
                    for g in range(NPIECE, NG):
                        emit_af(g)

                    # alpha_w^T columns (bf16 to match bf16 e tiles)
                    for hc in range(NHC):
                        ps = sps.tile([128, 1], F32, tag="aw")
                        nc.tensor.matmul(
                            ps[:],
                            aw_sb[0:1, hc * 128 : (hc + 1) * 128],
                            ones11[:],
                            start=True,
                            stop=True,
                        )
                        nc.vector.tensor_copy(awT[:, hc : hc + 1], ps[:])

                    # WahT[h, b] = sum_r Wah_w[h, r] * h[b, r]
                    wahT_ps = [
                        sacc.tile([128, BL], F32, tag=f"acc{hc}", name=f"wahT_ps{hc}")
                        for hc in range(NHC)
                    ]
                    for rc in range(NRC):
                        for hc in range(NHC):
                            nc.tensor.matmul(
                                wahT_ps[hc][:],
                                wwT[:, rc, hc * 128 : (hc + 1) * 128],
                                hT[:, rc, :],
                                start=(rc == 0),
                                stop=(rc == NRC - 1),
                            )
                    for hc in range(NHC):
                        nc.vector.tensor_copy(wahT[:, hc, :], wahT_ps[hc][:])

                # ---------------- streaming loop ----------------
                with (
                    tc.tile_pool(name="sc_ps", bufs=1, space=PSUM) as pool_sc,
                    tc.tile_pool(name="aT_ps", bufs=2, space=PSUM) as pool_aT,
                    tc.tile_pool(name="acc_ps", bufs=1, space=PSUM) as pool_acc,
                ):
                    acc = [
                        pool_acc.tile([BL, 512], F32, tag=f"acc{f}", name=f"acc{f}")
                        for f in range(NFQ)
                    ]
                    zps = pool_acc.tile([BL, 1], F32, tag="z", name="zps")

                    # chunk q's alpha values are complete after pair rdy[q]
                    ready = [[] for _ in range(NPAIR)]
                    for q in range(NCH):
                        rb = (128 * q + 127) // LP
                        ready[rb // 2].append(q)

                    for pr in range(NPAIR):
                        # -------- phase 1: scores for the pair --------
                        sc = pool_sc.tile([1, 2, L], F32, tag="sc")
                        for hc in range(NHC):
                            e_bf = pool_e.tile([128, 2, L], BF16, tag=f"e{hc}")
                            for jb in range(2):
                                b = 2 * pr + jb
                                pc, j = divmod(b, JPP)
                                nc.scalar.activation(
                                    e_bf[:, jb, :],
                                    pa_t[:, pc, j, hc, :],
                                    Tanh,
                                    bias=wahT[:, hc, b : b + 1],
                                )
                            nc.tensor.matmul(
                                sc[:],
                                awT[:, hc : hc + 1],
                                e_bf[:],
                                start=(hc == 0),
                                stop=(hc == NHC - 1),
                            )
                        for jb in range(2):
                            b = 2 * pr + jb
                            nc.scalar.activation(
                                expS[0:1, b * LP : b * LP + L], sc[0:1, jb, :], Exp
                            )

                        # -------- phase 2 for chunks completed by this pair --------
                        for q in ready[pr]:
                            aT = pool_aT.tile([128, 1], F32, tag="aT")
                            nc.tensor.matmul(
                                aT[:],
                                expS[0:1, 128 * q : 128 * q + 128],
                                ones11[:],
                                start=True,
                                stop=True,
                            )
                            # scatter into the block-diagonal weight column(s);
                            # clamp to real rows so [196, 224) pad garbage is
                            # never copied (aT_all pad rows stay memset-zero)
                            r0 = 128 * q
                            r = r0
                            while r < r0 + 128:
                                b = r // LP
                                seg_end = min(r0 + 128, b * LP + L)
                                for p0, p1 in _legal_pieces(
                                    r - r0, max(seg_end, r) - r0
                                ):
                                    nc.vector.tensor_copy(
                                        aT_all[p0:p1, q, b : b + 1],
                                        aT[p0:p1, 0:1],
                                    )
                                r = (b + 1) * LP
                            g, qq = divmod(q, GCH)
                            lhs = aT_all[:, q, :]
                            for f in range(NFQ):
                                nc.tensor.matmul(
                                    acc[f][:],
                                    lhs,
                                    af_t[g][:, qq, f * 512 : (f + 1) * 512],
                                    start=(q == 0),
                                    stop=(q == NCH - 1),
                                )
                            nc.tensor.matmul(
                                zps[:],
                                lhs,
                                onesb[:],
                                start=(q == 0),
                                stop=(q == NCH - 1),
                            )

                    # -------- normalize + store --------
                    # scale-copies split across DVE and ScalarE (both idle by
                    # now) and the output DMA goes out per f-bank so the last
                    # bank's copy overlaps the earlier banks' stores.
                    nc.vector.reciprocal(rz[:], zps[:])
                    for f in range(NFQ):
                        fsl = slice(f * 512, (f + 1) * 512)
                        if f % 2 == 0:
                            nc.vector.tensor_scalar_mul(
                                out_sb[:, fsl], acc[f][:], rz[:]
                            )
                        else:
                            nc.scalar.mul(out_sb[:, fsl], acc[f][:], rz[:])
                        nc.gpsimd.dma_start(out_d[:, fsl], out_sb[:, fsl])

    if split:
        _split_sync(nc)
    return nc


_NC_CACHE = None


def _get_nc():
    global _NC_CACHE
    if _NC_CACHE is None:
        _NC_CACHE = build_nc()
    return _NC_CACHE


def _make_in_maps(h, att_feats, p_att_feats, Wah_w, alpha_w):
    import ml_dtypes

    bf = ml_dtypes.bfloat16
    h = np.ascontiguousarray(h, dtype=np.float32)
    att_feats = np.ascontiguousarray(att_feats, dtype=np.float32)
    p_att_feats = np.ascontiguousarray(p_att_feats, dtype=np.float32)
    Wah_w = np.ascontiguousarray(Wah_w, dtype=np.float32)
    alpha_w = np.ascontiguousarray(alpha_w, dtype=np.float32)
    # Wah_w [HID, RNN] -> [128, NRC, HID]: element (p, rc, c) = W[c, 128*rc+p]
    wwT = np.ascontiguousarray(
        Wah_w.T.reshape(NRC, 128, HID).transpose(1, 0, 2).astype(bf)
    )
    in_maps = []
    for i in range(NCORES):
        sl = slice(i * BL, (i + 1) * BL)
        # att_feats -> r-major chunks (l padded to LP): [NG, 128, GCH, FEAT]
        af_pad = np.zeros((BL, LP, FEAT), dtype=bf)
        af_pad[:, :L] = att_feats[sl]
        af = af_pad.reshape(NG, GCH, 128, FEAT).transpose(0, 2, 1, 3)
        # p_att -> [NPIECE, 128, JPP, NHC, L] (h-major on partitions)
        pa = (
            p_att_feats[sl]
            .reshape(NPIECE, JPP, L, NHC, 128)
            .transpose(0, 4, 1, 3, 2)
            .astype(bf)
        )
        # h [BL, RNN] -> [128, NRC, BL]: element (p, rc, b) = h[b, 128*rc+p]
        hT = h[sl].T.reshape(NRC, 128, BL).transpose(1, 0, 2).astype(bf)
        in_maps.append(
            {
                "h": np.ascontiguousarray(hT),
                "att_feats": np.ascontiguousarray(af),
                "p_att_feats": np.ascontiguousarray(pa),
                "Wah_w": wwT,
                "alpha_w": alpha_w,
            }
        )
    return in_maps


def run_spmd(h, att_feats, p_att_feats, Wah_w, alpha_w, trace=False):
    """Run the SPMD kernel; returns (full_output, BassKernelResults)."""
    from concourse.bass_utils import run_bass_kernel_spmd

    nc = _get_nc()
    in_maps = _make_in_maps(h, att_feats, p_att_feats, Wah_w, alpha_w)
    res = run_bass_kernel_spmd(nc, in_maps, list(range(NCORES)), trace=trace)
    out = np.concatenate([res.results[i]["out"] for i in range(NCORES)], axis=0)
    return out, res


def kernel(h, att_feats, p_att_feats, Wah_w, alpha_w):
    out, _ = run_spmd(h, att_feats, p_att_feats, Wah_w, alpha_w, trace=False)
    return out
